# revision 7
# baseline (speedup 1.0000x reference)
"""Trainium2 Bass kernel for PointProp message passing + FC head.

Reference computation (per point n, K=8 components, D=E=256, H=132):
    right = (sum_k comp_k) @ Wm.T + K*bm
    right = right @ Wu.T + bu
    x     = [signal, right]                       # [N, 512]
    h     = relu-MLP(x; W0,W1,W2) ; out = h @ W3.T + b3

Key algebraic fold (host-side, fp64): `right` only enters through W0's
right half W0r, so
    A  = W0r @ Wu @ Wm                            # [132, 256]
    c0 = W0r @ (K*(Wu@bm) + bu) + b0              # [132]
    h0 = relu(signal @ W0s.T + (sum_k comp_k) @ A.T + c0)
which removes both DxD linear layers from the device kernel.

Layout: host pre-transposes signal and components shards to feature-major
([E, nsh], [K, D, nsh]) so the tensor engine needs no on-chip transposes;
all activations stay feature-major end to end and the output is stored
[D, nsh], transposed back on the host.

Device kernel (per core, data-parallel over N):
  - stream 512-point superblocks
  - K-sum on DVE (in-place add tree), final level writes fp32r
  - FC stack as fp32r matmuls (full PE rate), H=132 split into 128+4
  - relu+bias on ACT straight out of PSUM
"""

import numpy as np

import concourse.bacc as bacc
import concourse.bass as bass
import concourse.mybir as mybir
import concourse.tile as tile
from concourse.bass_utils import run_bass_kernel_spmd

F32 = mybir.dt.float32
F32R = mybir.dt.float32r

N_FULL = 65536
K = 8
D = 256
E = 256
H = 132
NCORES = 8
NSH = N_FULL // NCORES  # 8192 points per core
SBLK = 512              # points per superblock


def build_nc(nsh=NSH):
    """Build the single-core Bass program (same program runs SPMD on all cores)."""
    nblk = nsh // SBLK
    nc = bacc.Bacc("TRN2", target_bir_lowering=False, debug=False)

    comp = nc.declare_dram_parameter("compT", [K, D, nsh], F32, isOutput=False)
    sig = nc.declare_dram_parameter("sigT", [E, nsh], F32R, isOutput=False)
    w0 = nc.declare_dram_parameter("w0t", [E + D, H], F32R, isOutput=False)
    w1 = nc.declare_dram_parameter("w1t", [H, H], F32R, isOutput=False)
    w2 = nc.declare_dram_parameter("w2t", [H, H], F32R, isOutput=False)
    w3 = nc.declare_dram_parameter("w3t", [H, D], F32R, isOutput=False)
    c0 = nc.declare_dram_parameter("c0", [H, 1], F32, isOutput=False)
    c1 = nc.declare_dram_parameter("c1", [H, 1], F32, isOutput=False)
    c2 = nc.declare_dram_parameter("c2", [H, 1], F32, isOutput=False)
    c3 = nc.declare_dram_parameter("c3", [D, 1], F32, isOutput=False)
    outT = nc.declare_dram_parameter("outT", [D, nsh], F32, isOutput=True)

    # compT[k, (ch p), (s t)] -> [s, ch, k, p, t]
    comp_v = comp.ap().rearrange("k (c p) (s t) -> s c p k t", p=128, t=SBLK)
    sig_v = sig.ap().rearrange("(c p) (s t) -> s p c t", p=128, t=SBLK)
    outT_v = outT.ap()

    def mm(out, lhsT, rhs, start, stop):
        nc.tensor.matmul(
            out, lhsT.bitcast(F32R), rhs.bitcast(F32R), start=start, stop=stop
        )

    with tile.TileContext(nc) as tc:
        with (
            tc.tile_pool(name="const", bufs=1) as cpool,
            tc.tile_pool(name="comp", bufs=3) as comp_pool,
            tc.tile_pool(name="sig", bufs=3) as sig_pool,
            tc.tile_pool(name="cst", bufs=2) as cs_pool,
            tc.tile_pool(name="hsb", bufs=2) as h_pool,
            tc.tile_pool(name="osb", bufs=3) as o_pool,
            tc.tile_pool(name="hpsum", bufs=4, space="PSUM") as hpsum,
            tc.tile_pool(name="qpsum", bufs=2, space="PSUM") as qpsum,
        ):
            w0t = cpool.tile([128, 4, H], F32R)
            nc.sync.dma_start(w0t, w0.ap().rearrange("(c p) m -> p c m", p=128))
            w1a = cpool.tile([128, H], F32R)
            nc.sync.dma_start(w1a, w1.ap()[0:128, :])
            w1b = cpool.tile([4, H], F32R)
            nc.sync.dma_start(w1b, w1.ap()[128:H, :])
            w2a = cpool.tile([128, H], F32R)
            nc.sync.dma_start(w2a, w2.ap()[0:128, :])
            w2b = cpool.tile([4, H], F32R)
            nc.sync.dma_start(w2b, w2.ap()[128:H, :])
            w3a = cpool.tile([128, D], F32R)
            nc.sync.dma_start(w3a, w3.ap()[0:128, :])
            w3b = cpool.tile([4, D], F32R)
            nc.sync.dma_start(w3b, w3.ap()[128:H, :])

            c0a = cpool.tile([128, 1], F32)
            nc.sync.dma_start(c0a, c0.ap()[0:128, :])
            c0b = cpool.tile([4, 1], F32)
            nc.sync.dma_start(c0b, c0.ap()[128:H, :])
            c1a = cpool.tile([128, 1], F32)
            nc.sync.dma_start(c1a, c1.ap()[0:128, :])
            c1b = cpool.tile([4, 1], F32)
            nc.sync.dma_start(c1b, c1.ap()[128:H, :])
            c2a = cpool.tile([128, 1], F32)
            nc.sync.dma_start(c2a, c2.ap()[0:128, :])
            c2b = cpool.tile([4, 1], F32)
            nc.sync.dma_start(c2b, c2.ap()[128:H, :])
            c3t = cpool.tile([128, 2], F32)
            nc.sync.dma_start(c3t, c3.ap().rearrange("(c p) o -> p (c o)", p=128))

            relu = mybir.ActivationFunctionType.Relu
            idf = mybir.ActivationFunctionType.Identity

            for s in range(nblk):
                # ---- load (all feature-major) ----
                ct = comp_pool.tile([128, 2, K, SBLK], F32, tag="ct")
                for ch in range(2):
                    nc.sync.dma_start(ct[:, ch, :, :], comp_v[s, ch])
                st = sig_pool.tile([128, 2, SBLK], F32R, tag="st")
                nc.sync.dma_start(st, sig_v[s])

                # ---- K-sum tree on DVE; final level casts to fp32r ----
                nc.vector.tensor_add(
                    ct[:, :, 0:4, :], ct[:, :, 0:4, :], ct[:, :, 4:8, :]
                )
                nc.vector.tensor_add(
                    ct[:, :, 0:2, :], ct[:, :, 0:2, :], ct[:, :, 2:4, :]
                )
                cs = cs_pool.tile([128, 2, SBLK], F32R, tag="cs")
                nc.vector.tensor_add(cs, ct[:, :, 0, :], ct[:, :, 1, :])

                xT = [st[:, 0, :], st[:, 1, :], cs[:, 0, :], cs[:, 1, :]]

                # ---- layer 0: h0 = relu(W0cat^T.T @ xT + c0), H split 128+4 ----
                h0p = hpsum.tile([128, SBLK], F32, tag="hp")
                for ch in range(4):
                    mm(h0p, w0t[:, ch, 0:128], xT[ch], ch == 0, ch == 3)
                h0q = qpsum.tile([4, SBLK], F32, tag="hq")
                for ch in range(4):
                    mm(h0q, w0t[:, ch, 128:H], xT[ch], ch == 0, ch == 3)
                h0a = h_pool.tile([128, SBLK], F32R, tag="h0a")
                nc.scalar.activation(h0a, h0p, relu, bias=c0a)
                h0b = h_pool.tile([4, SBLK], F32R, tag="h0b")
                nc.scalar.activation(h0b, h0q, relu, bias=c0b)

                # ---- layer 1 ----
                h1p = hpsum.tile([128, SBLK], F32, tag="hp")
                mm(h1p, w1a[:, 0:128], h0a, True, False)
                mm(h1p, w1b[:, 0:128], h0b, False, True)
                h1q = qpsum.tile([4, SBLK], F32, tag="hq")
                mm(h1q, w1a[:, 128:H], h0a, True, False)
                mm(h1q, w1b[:, 128:H], h0b, False, True)
                h1a = h_pool.tile([128, SBLK], F32R, tag="h1a")
                nc.scalar.activation(h1a, h1p, relu, bias=c1a)
                h1b = h_pool.tile([4, SBLK], F32R, tag="h1b")
                nc.scalar.activation(h1b, h1q, relu, bias=c1b)

                # ---- layer 2 ----
                h2p = hpsum.tile([128, SBLK], F32, tag="hp")
                mm(h2p, w2a[:, 0:128], h1a, True, False)
                mm(h2p, w2b[:, 0:128], h1b, False, True)
                h2q = qpsum.tile([4, SBLK], F32, tag="hq")
                mm(h2q, w2a[:, 128:H], h1a, True, False)
                mm(h2q, w2b[:, 128:H], h1b, False, True)
                h2a = h_pool.tile([128, SBLK], F32R, tag="h2a")
                nc.scalar.activation(h2a, h2p, relu, bias=c2a)
                h2b = h_pool.tile([4, SBLK], F32R, tag="h2b")
                nc.scalar.activation(h2b, h2q, relu, bias=c2b)

                # ---- layer 3: out^T = W3^T.T @ h2 + b3, D=256 in two 128 halves ----
                for half in range(2):
                    op = hpsum.tile([128, SBLK], F32, tag="hp")
                    mm(op, w3a[:, half * 128 : (half + 1) * 128], h2a, True, False)
                    mm(op, w3b[:, half * 128 : (half + 1) * 128], h2b, False, True)
                    ot = o_pool.tile([128, SBLK], F32, tag=f"ot{half}")
                    nc.scalar.activation(ot, op, idf, bias=c3t[:, half : half + 1])
                    nc.sync.dma_start(
                        outT_v[half * 128 : (half + 1) * 128, s * SBLK : (s + 1) * SBLK],
                        ot,
                    )

    nc.compile()
    return nc


def fold_weights(Wm, bm, Wu, bu, W0, b0, W1, b1, W2, b2, W3, b3, k):
    f8 = np.float64
    W0s = W0[:, :E].astype(f8)
    W0r = W0[:, E:].astype(f8)
    A = W0r @ Wu.astype(f8) @ Wm.astype(f8)
    c0 = W0r @ (k * (Wu.astype(f8) @ bm.astype(f8)) + bu.astype(f8)) + b0.astype(f8)
    w0t = np.ascontiguousarray(
        np.concatenate([W0s, A], axis=1).T.astype(np.float32)
    )  # [E+D, H]
    return {
        "w0t": w0t,
        "w1t": np.ascontiguousarray(W1.T.astype(np.float32)),
        "w2t": np.ascontiguousarray(W2.T.astype(np.float32)),
        "w3t": np.ascontiguousarray(W3.T.astype(np.float32)),
        "c0": np.ascontiguousarray(c0.astype(np.float32)[:, None]),
        "c1": np.ascontiguousarray(b1.astype(np.float32)[:, None]),
        "c2": np.ascontiguousarray(b2.astype(np.float32)[:, None]),
        "c3": np.ascontiguousarray(b3.astype(np.float32)[:, None]),
    }


_NC_CACHE = {}


def _get_nc(nsh=NSH):
    if nsh not in _NC_CACHE:
        _NC_CACHE[nsh] = build_nc(nsh)
    return _NC_CACHE[nsh]


def make_in_maps(signal, components, wmap):
    """Shard + relayout (feature-major) the full inputs for the 8 cores."""
    signal = np.asarray(signal, dtype=np.float32)
    components = np.asarray(components, dtype=np.float32)
    sigT = np.ascontiguousarray(signal.T)  # [E, N]
    in_maps = []
    for i in range(NCORES):
        lo, hi = i * NSH, (i + 1) * NSH
        m = dict(wmap)
        m["compT"] = np.ascontiguousarray(components[:, lo:hi, :].transpose(0, 2, 1))
        m["sigT"] = np.ascontiguousarray(sigT[:, lo:hi])
        in_maps.append(m)
    return in_maps


def kernel(signal, components, Wm, bm, Wu, bu, W0, b0, W1, b1, W2, b2, W3, b3):
    wmap = fold_weights(Wm, bm, Wu, bu, W0, b0, W1, b1, W2, b2, W3, b3, K)
    nc = _get_nc()
    in_maps = make_in_maps(signal, components, wmap)
    res = run_bass_kernel_spmd(nc, in_maps, core_ids=list(range(NCORES)))
    out = np.concatenate(
        [np.asarray(r["outT"]).T for r in res.results], axis=0
    )
    return np.ascontiguousarray(out.astype(np.float32))


# revision 8
# speedup vs baseline: 1.2285x; 1.2285x over previous
"""Trainium2 Bass kernel for PointProp message passing + FC head.

Reference computation (per point n, K=8 components, D=E=256, H=132):
    right = (sum_k comp_k) @ Wm.T + K*bm
    right = right @ Wu.T + bu
    x     = [signal, right]                       # [N, 512]
    h     = relu-MLP(x; W0,W1,W2) ; out = h @ W3.T + b3

Key algebraic fold (host-side, fp64): `right` only enters through W0's
right half W0r, so
    A  = W0r @ Wu @ Wm                            # [132, 256]
    c0 = W0r @ (K*(Wu@bm) + bu) + b0              # [132]
    h0 = relu(signal @ W0s.T + (sum_k comp_k) @ A.T + c0)
which removes both DxD linear layers from the device kernel.

Layout: host pre-transposes signal and components shards to feature-major
([E, nsh], [K, D, nsh]) so the tensor engine needs no on-chip transposes;
all activations stay feature-major end to end and the output is stored
[D, nsh], transposed back on the host.

Device kernel (per core, data-parallel over N):
  - stream 512-point superblocks
  - K-sum on DVE (in-place add tree), final level writes fp32r
  - FC stack as fp32r matmuls (full PE rate), H=132 split into 128+4
  - relu+bias on ACT straight out of PSUM
"""

import numpy as np

import concourse.bacc as bacc
import concourse.bass as bass
import concourse.mybir as mybir
import concourse.tile as tile
from concourse.bass_utils import run_bass_kernel_spmd

F32 = mybir.dt.float32
F32R = mybir.dt.float32r

N_FULL = 65536
K = 8
D = 256
E = 256
H = 132
NCORES = 8
NSH = N_FULL // NCORES  # 8192 points per core
SBLK = 512              # points per superblock


def build_nc(nsh=NSH):
    """Build the single-core Bass program (same program runs SPMD on all cores)."""
    nblk = nsh // SBLK
    nc = bacc.Bacc("TRN2", target_bir_lowering=False, debug=False)

    comp = nc.declare_dram_parameter("compR", [nblk, 128, 2 * K * SBLK], F32, isOutput=False)
    sig = nc.declare_dram_parameter("sigR", [nblk, 128, 2 * SBLK], F32R, isOutput=False)
    w0 = nc.declare_dram_parameter("w0t", [E + D, H], F32R, isOutput=False)
    w1 = nc.declare_dram_parameter("w1t", [H, H], F32R, isOutput=False)
    w2 = nc.declare_dram_parameter("w2t", [H, H], F32R, isOutput=False)
    w3 = nc.declare_dram_parameter("w3t", [H, D], F32R, isOutput=False)
    c0 = nc.declare_dram_parameter("c0", [H, 1], F32, isOutput=False)
    c1 = nc.declare_dram_parameter("c1", [H, 1], F32, isOutput=False)
    c2 = nc.declare_dram_parameter("c2", [H, 1], F32, isOutput=False)
    c3 = nc.declare_dram_parameter("c3", [D, 1], F32, isOutput=False)
    outR = nc.declare_dram_parameter("outR", [nblk, 2, 128, SBLK], F32, isOutput=True)

    comp_v = comp.ap()
    sig_v = sig.ap()
    outR_v = outR.ap()

    def mm(out, lhsT, rhs, start, stop):
        nc.tensor.matmul(
            out, lhsT.bitcast(F32R), rhs.bitcast(F32R), start=start, stop=stop
        )

    with tile.TileContext(nc) as tc:
        with (
            tc.tile_pool(name="const", bufs=1) as cpool,
            tc.tile_pool(name="comp", bufs=3) as comp_pool,
            tc.tile_pool(name="sig", bufs=3) as sig_pool,
            tc.tile_pool(name="cst", bufs=2) as cs_pool,
            tc.tile_pool(name="hsb", bufs=2) as h_pool,
            tc.tile_pool(name="osb", bufs=3) as o_pool,
            tc.tile_pool(name="hpsum", bufs=4, space="PSUM") as hpsum,
            tc.tile_pool(name="qpsum", bufs=2, space="PSUM") as qpsum,
        ):
            w0t = cpool.tile([128, 4, H], F32R)
            nc.sync.dma_start(w0t, w0.ap().rearrange("(c p) m -> p c m", p=128))
            w1a = cpool.tile([128, H], F32R)
            nc.sync.dma_start(w1a, w1.ap()[0:128, :])
            w1b = cpool.tile([4, H], F32R)
            nc.sync.dma_start(w1b, w1.ap()[128:H, :])
            w2a = cpool.tile([128, H], F32R)
            nc.sync.dma_start(w2a, w2.ap()[0:128, :])
            w2b = cpool.tile([4, H], F32R)
            nc.sync.dma_start(w2b, w2.ap()[128:H, :])
            w3a = cpool.tile([128, D], F32R)
            nc.sync.dma_start(w3a, w3.ap()[0:128, :])
            w3b = cpool.tile([4, D], F32R)
            nc.sync.dma_start(w3b, w3.ap()[128:H, :])

            c0a = cpool.tile([128, 1], F32)
            nc.sync.dma_start(c0a, c0.ap()[0:128, :])
            c0b = cpool.tile([4, 1], F32)
            nc.sync.dma_start(c0b, c0.ap()[128:H, :])
            c1a = cpool.tile([128, 1], F32)
            nc.sync.dma_start(c1a, c1.ap()[0:128, :])
            c1b = cpool.tile([4, 1], F32)
            nc.sync.dma_start(c1b, c1.ap()[128:H, :])
            c2a = cpool.tile([128, 1], F32)
            nc.sync.dma_start(c2a, c2.ap()[0:128, :])
            c2b = cpool.tile([4, 1], F32)
            nc.sync.dma_start(c2b, c2.ap()[128:H, :])
            c3t = cpool.tile([128, 2], F32)
            nc.sync.dma_start(c3t, c3.ap().rearrange("(c p) o -> p (c o)", p=128))

            relu = mybir.ActivationFunctionType.Relu
            idf = mybir.ActivationFunctionType.Identity

            for s in range(nblk):
                # ---- load (all feature-major) ----
                ct = comp_pool.tile([128, 2, K, SBLK], F32, tag="ct")
                nc.sync.dma_start(ct, comp_v[s])
                st = sig_pool.tile([128, 2, SBLK], F32R, tag="st")
                nc.sync.dma_start(st, sig_v[s])

                # ---- K-sum tree on DVE; final level casts to fp32r ----
                nc.vector.tensor_add(
                    ct[:, :, 0:4, :], ct[:, :, 0:4, :], ct[:, :, 4:8, :]
                )
                nc.vector.tensor_add(
                    ct[:, :, 0:2, :], ct[:, :, 0:2, :], ct[:, :, 2:4, :]
                )
                cs = cs_pool.tile([128, 2, SBLK], F32R, tag="cs")
                nc.vector.tensor_add(cs, ct[:, :, 0, :], ct[:, :, 1, :])

                xT = [st[:, 0, :], st[:, 1, :], cs[:, 0, :], cs[:, 1, :]]

                # ---- layer 0: h0 = relu(W0cat^T.T @ xT + c0), H split 128+4 ----
                h0p = hpsum.tile([128, SBLK], F32, tag="hp")
                for ch in range(4):
                    mm(h0p, w0t[:, ch, 0:128], xT[ch], ch == 0, ch == 3)
                h0q = qpsum.tile([4, SBLK], F32, tag="hq")
                for ch in range(4):
                    mm(h0q, w0t[:, ch, 128:H], xT[ch], ch == 0, ch == 3)
                h0a = h_pool.tile([128, SBLK], F32R, tag="h0a")
                nc.scalar.activation(h0a, h0p, relu, bias=c0a)
                h0b = h_pool.tile([4, SBLK], F32R, tag="h0b")
                nc.scalar.activation(h0b, h0q, relu, bias=c0b)

                # ---- layer 1 ----
                h1p = hpsum.tile([128, SBLK], F32, tag="hp")
                mm(h1p, w1a[:, 0:128], h0a, True, False)
                mm(h1p, w1b[:, 0:128], h0b, False, True)
                h1q = qpsum.tile([4, SBLK], F32, tag="hq")
                mm(h1q, w1a[:, 128:H], h0a, True, False)
                mm(h1q, w1b[:, 128:H], h0b, False, True)
                h1a = h_pool.tile([128, SBLK], F32R, tag="h1a")
                nc.scalar.activation(h1a, h1p, relu, bias=c1a)
                h1b = h_pool.tile([4, SBLK], F32R, tag="h1b")
                nc.scalar.activation(h1b, h1q, relu, bias=c1b)

                # ---- layer 2 ----
                h2p = hpsum.tile([128, SBLK], F32, tag="hp")
                mm(h2p, w2a[:, 0:128], h1a, True, False)
                mm(h2p, w2b[:, 0:128], h1b, False, True)
                h2q = qpsum.tile([4, SBLK], F32, tag="hq")
                mm(h2q, w2a[:, 128:H], h1a, True, False)
                mm(h2q, w2b[:, 128:H], h1b, False, True)
                h2a = h_pool.tile([128, SBLK], F32R, tag="h2a")
                nc.scalar.activation(h2a, h2p, relu, bias=c2a)
                h2b = h_pool.tile([4, SBLK], F32R, tag="h2b")
                nc.scalar.activation(h2b, h2q, relu, bias=c2b)

                # ---- layer 3: out^T = W3^T.T @ h2 + b3, D=256 in two 128 halves ----
                for half in range(2):
                    op = hpsum.tile([128, SBLK], F32, tag="hp")
                    mm(op, w3a[:, half * 128 : (half + 1) * 128], h2a, True, False)
                    mm(op, w3b[:, half * 128 : (half + 1) * 128], h2b, False, True)
                    ot = o_pool.tile([128, SBLK], F32, tag=f"ot{half}")
                    nc.scalar.activation(ot, op, idf, bias=c3t[:, half : half + 1])
                    nc.sync.dma_start(outR_v[s, half], ot)

    nc.compile()
    return nc


def fold_weights(Wm, bm, Wu, bu, W0, b0, W1, b1, W2, b2, W3, b3, k):
    f8 = np.float64
    W0s = W0[:, :E].astype(f8)
    W0r = W0[:, E:].astype(f8)
    A = W0r @ Wu.astype(f8) @ Wm.astype(f8)
    c0 = W0r @ (k * (Wu.astype(f8) @ bm.astype(f8)) + bu.astype(f8)) + b0.astype(f8)
    w0t = np.ascontiguousarray(
        np.concatenate([W0s, A], axis=1).T.astype(np.float32)
    )  # [E+D, H]
    return {
        "w0t": w0t,
        "w1t": np.ascontiguousarray(W1.T.astype(np.float32)),
        "w2t": np.ascontiguousarray(W2.T.astype(np.float32)),
        "w3t": np.ascontiguousarray(W3.T.astype(np.float32)),
        "c0": np.ascontiguousarray(c0.astype(np.float32)[:, None]),
        "c1": np.ascontiguousarray(b1.astype(np.float32)[:, None]),
        "c2": np.ascontiguousarray(b2.astype(np.float32)[:, None]),
        "c3": np.ascontiguousarray(b3.astype(np.float32)[:, None]),
    }


_NC_CACHE = {}


def _get_nc(nsh=NSH):
    if nsh not in _NC_CACHE:
        _NC_CACHE[nsh] = build_nc(nsh)
    return _NC_CACHE[nsh]


def pack_comp(shard):
    """[K, nsh, D] -> [nblk, 128, 2*K*SBLK]: per-superblock tile bytes, contiguous."""
    k, nsh, d = shard.shape
    nblk = nsh // SBLK
    v = shard.reshape(k, nblk, SBLK, 2, 128)          # [k, s, t, ch, p]
    v = v.transpose(1, 4, 3, 0, 2)                    # [s, p, ch, k, t]
    return np.ascontiguousarray(v).reshape(nblk, 128, 2 * K * SBLK)


def pack_sig(shard):
    """[nsh, E] -> [nblk, 128, 2*SBLK]."""
    nsh, e = shard.shape
    nblk = nsh // SBLK
    v = shard.reshape(nblk, SBLK, 2, 128)             # [s, t, ch, p]
    v = v.transpose(0, 3, 2, 1)                       # [s, p, ch, t]
    return np.ascontiguousarray(v).reshape(nblk, 128, 2 * SBLK)


def unpack_out(outR):
    """[nblk, 2, 128, SBLK] -> [nsh, D]."""
    nblk = outR.shape[0]
    v = outR.transpose(0, 3, 1, 2)                    # [s, t, half, j]
    return np.ascontiguousarray(v).reshape(nblk * SBLK, D)


def make_in_maps(signal, components, wmap):
    """Shard + repack the full inputs for the 8 cores."""
    signal = np.asarray(signal, dtype=np.float32)
    components = np.asarray(components, dtype=np.float32)
    in_maps = []
    for i in range(NCORES):
        lo, hi = i * NSH, (i + 1) * NSH
        m = dict(wmap)
        m["compR"] = pack_comp(components[:, lo:hi, :])
        m["sigR"] = pack_sig(signal[lo:hi, :])
        in_maps.append(m)
    return in_maps


def kernel(signal, components, Wm, bm, Wu, bu, W0, b0, W1, b1, W2, b2, W3, b3):
    wmap = fold_weights(Wm, bm, Wu, bu, W0, b0, W1, b1, W2, b2, W3, b3, K)
    nc = _get_nc()
    in_maps = make_in_maps(signal, components, wmap)
    res = run_bass_kernel_spmd(nc, in_maps, core_ids=list(range(NCORES)))
    out = np.concatenate(
        [unpack_out(np.asarray(r["outR"])) for r in res.results], axis=0
    )
    return np.ascontiguousarray(out.astype(np.float32))
